# revision 3
# baseline (speedup 1.0000x reference)
"""Trainium2 Bass kernel for GaussianProcessEmbeddingHead.

The reference computes:
    mu     = x @ W_mu.T + b_mu                      (B,N,E)
    sigma  = exp(0.5*(x @ W_logvar.T + b_logvar))   (B,N,E)
    K      = RBF kernel matrix (B,N,N)  -- only its DIAGONAL is used,
             and dist_ii == 0 exactly, so cov_diag == 1 and the (B,N,N)
             work is mathematically dead. sigma_adjusted == sigma.
    return (mu, sigma_adjusted)

Strategy: data-parallel over batch B=8, one batch element per NeuronCore.
Per core: two linear heads over x_b [2048,1024] in bf16. The PE streams
one output column per cycle, so the floor is
   2 heads * (2048*512 outputs / 128 lanes) * (1024/128 k-tiles)
   = 131072 cycles ~= 54.6 us @ 2.4 GHz.

Schedule (v2): the old schedule idled the PE for ~9.5 us at the start
waiting for full-chunk SWDGE loads, and the warmup->stream gap (6.5 us)
re-throttled the HAM clock gate so the first ~12 real matmuls ran cold.
Now:
 - Critical-path loads are slab-granular on the two fast HWDGE queues:
   sync carries x chunk 0 in 4 pairs of k-tiles (256 KB each), scalar
   carries W_logvar in 4 matching pairs. First matmul issues ~2.4 us
   into exec; each kt-step consumes exactly one x slab + one w slab at
   the rate the rings deliver them.
 - Chunks run kt-OUTER / eb-inner with 4 PSUM banks accumulating in
   parallel, so compute needs only the k-slabs that have arrived, not
   the whole chunk. The final chunk reverts to eb-outer with a tapered
   last group (256/128/128) so the serialized end-of-kernel epilogue is
   short.
 - Warmup matmuls abut the real stream (PE busy continuously from
   ~0.2 us), so HAM flips to 2.4 GHz ~3.4 us in and stays there.
 - Remaining loads ride the free ring capacity: x_c1 split across
   sync+scalar right behind the critical slabs, x_c2/x_c3 + biases on
   gpsimd (SWDGE fixed cost hidden), wmu on scalar. lv stores ride
   sync, mu stores ride scalar.
 - Outputs are produced transposed ([E, N], partition = embedding), so
   each PSUM tile needs exactly ONE epilogue op with the bias fed
   through the per-partition port:
     sigma = Exp(PSUM * 0.5 + 0.5*b_lv[e])  on the Scalar engine
     mu    = PSUM + b_mu[e]                 on the Vector engine
   both writing bf16; host un-transposes and upcasts.
"""
import os
import sys

import numpy as np

try:
    import concourse.bass as bass  # noqa: F401
except Exception:  # pragma: no cover - path fallback for fresh dirs
    for p in ("/opt/trn_rl_repo", os.path.expanduser("~/.axon_site/_ro/trn_rl_repo")):
        if os.path.isdir(p) and p not in sys.path:
            sys.path.insert(0, p)
    import concourse.bass as bass

import ml_dtypes
import concourse.mybir as mybir
from concourse import bacc
from concourse.bass_utils import run_bass_kernel_spmd
from concourse.tile import TileContext

B, N, D, E = 8, 2048, 1024, 512
P = 128
KT = D // P          # 8 k-tiles
EB = E // P          # 4 embedding blocks
TC = N // 512        # 4 token chunks of 512
F32, BF16 = mybir.dt.float32, mybir.dt.bfloat16

_NC = None


def _build():
    nc = bacc.Bacc()
    # x packed on host as [p][c][kt][t] -> [P, KT*N]
    xP = nc.declare_dram_parameter("xP", [P, KT * N], BF16, isOutput=False)
    # weights packed as [p][kt][e] -> [P, KT*E]
    wlv = nc.declare_dram_parameter("wlv", [P, E * KT], BF16, isOutput=False)
    wmu = nc.declare_dram_parameter("wmu", [P, E * KT], BF16, isOutput=False)
    # biases arranged [P, EB]: element (p, eb) = bias[eb*128 + p]
    bmu = nc.declare_dram_parameter("bmu", [P, EB], F32, isOutput=False)
    blv = nc.declare_dram_parameter("blv", [P, EB], F32, isOutput=False)  # 0.5*b
    muT = nc.declare_dram_parameter("muT", [E, N], BF16, isOutput=True)
    sgT = nc.declare_dram_parameter("sgT", [E, N], BF16, isOutput=True)

    with TileContext(nc) as tc:
        with (
            tc.tile_pool(name="const", bufs=1) as cpool,
            tc.tile_pool(name="out", bufs=6) as opool,
            tc.tile_pool(name="psA", bufs=4, space="PSUM") as psA,
            tc.tile_pool(name="psB", bufs=4, space="PSUM") as psB,
        ):
            x_sb = [
                cpool.tile([P, KT, 512], BF16, name=f"x_sb{c}") for c in range(TC)
            ]
            wlv_sb = cpool.tile([P, KT, E], BF16)
            wmu_sb = cpool.tile([P, KT, E], BF16)
            blv_sb = cpool.tile([P, EB], F32)
            bmu_sb = cpool.tile([P, EB], F32)
            warm = cpool.tile([P, P], BF16)

            wlv_r = wlv[:, :].rearrange("p (kt e) -> p kt e", kt=KT)
            wmu_r = wmu[:, :].rearrange("p (kt e) -> p kt e", kt=KT)

            def xslab(c):
                off = c * 512 * KT
                return xP[:, off : off + 512 * KT].rearrange(
                    "p (kt t) -> p kt t", kt=KT
                )

            # Warmup: PE busy continuously from ~0.2us so the HAM clock
            # gate flips to 2.4 GHz ~3.4us in with no re-throttle gap.
            nc.vector.memset(warm, 0)
            wps = psA.tile([P, P], F32, tag="ps", name="warmps")
            for i in range(20):
                nc.tensor.matmul(
                    wps, warm[:, :], warm[:, :], start=(i == 0), stop=(i == 19)
                )

            # --- DMA schedule ---------------------------------------
            # sync (HWDGE):   x_c0 in 4 kt-pairs, then x_c1 kt0-3,
            #                 then lv stores (emitted inside the loop).
            # scalar (HWDGE): wlv in 4 kt-pairs, then x_c1 kt4-7, wmu,
            #                 then mu stores (emitted inside the loop).
            # gpsimd (SWDGE): biases, x_c2, x_c3 — all off the critical
            #                 path; the ~2us SWDGE fixed cost is hidden.
            for kp in range(4):
                nc.sync.dma_start(
                    out=x_sb[0][:, 2 * kp : 2 * kp + 2, :],
                    in_=xslab(0)[:, 2 * kp : 2 * kp + 2, :],
                )
                nc.scalar.dma_start(
                    out=wlv_sb[:, 2 * kp : 2 * kp + 2, :],
                    in_=wlv_r[:, 2 * kp : 2 * kp + 2, :],
                )
            nc.sync.dma_start(out=x_sb[1][:, 0:4, :], in_=xslab(1)[:, 0:4, :])
            nc.scalar.dma_start(out=x_sb[1][:, 4:KT, :], in_=xslab(1)[:, 4:KT, :])
            nc.gpsimd.dma_start(out=blv_sb, in_=blv[:, :])
            nc.gpsimd.dma_start(out=bmu_sb, in_=bmu[:, :])
            nc.gpsimd.dma_start(out=x_sb[2], in_=xslab(2))
            nc.gpsimd.dma_start(out=x_sb[3], in_=xslab(3))
            nc.scalar.dma_start(out=wmu_sb, in_=wmu_r[:, :, :])

            EXP = mybir.ActivationFunctionType.Exp

            def epilogue(hname, outdram, bias_sb, c, eb, ps, o0, ow):
                cs = slice(c * 512 + o0, c * 512 + o0 + ow)
                es = slice(eb * P, (eb + 1) * P)
                o = opool.tile([P, ow], BF16, tag="o", name=f"o_{hname}{c}{eb}_{o0}")
                if hname == "lv":
                    nc.scalar.activation(
                        o, ps, EXP, bias=bias_sb[:, eb : eb + 1], scale=0.5
                    )
                    nc.sync.dma_start(out=outdram[es, cs], in_=o)
                else:
                    nc.vector.tensor_scalar_add(o, ps, bias_sb[:, eb : eb + 1])
                    nc.scalar.dma_start(out=outdram[es, cs], in_=o)

            def chunk_ktouter(hname, w_sb, outdram, bias_sb, c, pool):
                """One token chunk, kt-outer: 4 PSUM banks accumulate in
                parallel; each kt step consumes one x slab + one w slab."""
                pss = [
                    pool.tile([P, 512], F32, tag="ps", name=f"ps_{hname}{c}{eb}")
                    for eb in range(EB)
                ]
                for kt in range(KT):
                    for eb in range(EB):
                        nc.tensor.matmul(
                            pss[eb],
                            w_sb[:, kt, eb * P : (eb + 1) * P],
                            x_sb[c][:, kt, :],
                            start=(kt == 0),
                            stop=(kt == KT - 1),
                        )
                for eb in range(EB):
                    epilogue(hname, outdram, bias_sb, c, eb, pss[eb], 0, 512)

            def group_ebouter(hname, w_sb, outdram, bias_sb, c, eb, o0, ow, pool):
                """Baseline-style group: kt-inner over columns [o0:o0+ow)."""
                es = slice(eb * P, (eb + 1) * P)
                ps = pool.tile([P, ow], F32, tag="ps", name=f"ps_{hname}{c}{eb}_{o0}")
                for kt in range(KT):
                    nc.tensor.matmul(
                        ps,
                        w_sb[:, kt, es],
                        x_sb[c][:, kt, o0 : o0 + ow],
                        start=(kt == 0),
                        stop=(kt == KT - 1),
                    )
                epilogue(hname, outdram, bias_sb, c, eb, ps, o0, ow)

            pools = [psA, psB]
            ci = 0
            for hname, w_sb, outdram, bias_sb in [
                ("lv", wlv_sb, sgT, blv_sb),
                ("mu", wmu_sb, muT, bmu_sb),
            ]:
                for c in range(TC):
                    last = hname == "mu" and c == TC - 1
                    if not last:
                        chunk_ktouter(hname, w_sb, outdram, bias_sb, c, pools[ci % 2])
                        ci += 1
                    else:
                        # Final chunk: eb-outer with tapered last group so
                        # the serialized end-of-kernel epilogue is short.
                        pool = pools[ci % 2]
                        for eb in range(EB - 1):
                            group_ebouter(
                                hname, w_sb, outdram, bias_sb, c, eb, 0, 512, pool
                            )
                        for o0, ow in [(0, 256), (256, 128), (384, 128)]:
                            group_ebouter(
                                hname, w_sb, outdram, bias_sb, c, EB - 1, o0, ow, pool
                            )
    nc.compile()
    return nc


def _pack_x(xb):
    """xb [N, D] f32 -> [P, KT*N] bf16 packed as [p][c][kt][t]."""
    xt = xb.T.astype(ml_dtypes.bfloat16).reshape(KT, P, TC, 512)  # [kt, p, c, t]
    return np.ascontiguousarray(xt.transpose(1, 2, 0, 3).reshape(P, KT * N))


def _pack_w(W):
    """W [E, D] f32 -> [P, KT*E] bf16 packed as [p][kt][e]."""
    wt = W.astype(np.float32).T.astype(ml_dtypes.bfloat16)   # [D, E]
    v = wt.reshape(KT, P, E)
    return np.ascontiguousarray(v.transpose(1, 0, 2).reshape(P, KT * E))


def run(x, W_mu, b_mu, W_logvar, b_logvar, trace=False, **trace_kwargs):
    global _NC
    if _NC is None:
        _NC = _build()

    x = np.asarray(x, dtype=np.float32)
    wlv_h = _pack_w(np.asarray(W_logvar))
    wmu_h = _pack_w(np.asarray(W_mu))
    bmu_h = np.ascontiguousarray(np.asarray(b_mu, dtype=np.float32).reshape(EB, P).T)
    blv_h = np.ascontiguousarray(
        (0.5 * np.asarray(b_logvar, dtype=np.float32)).reshape(EB, P).T
    )

    in_maps = [
        {
            "xP": _pack_x(x[b]),
            "wlv": wlv_h,
            "wmu": wmu_h,
            "bmu": bmu_h,
            "blv": blv_h,
        }
        for b in range(B)
    ]
    res = run_bass_kernel_spmd(
        _NC, in_maps, core_ids=list(range(B)), trace=trace, **trace_kwargs
    )
    mu = np.stack(
        [res.results[b]["muT"].reshape(E, N).T.astype(np.float32) for b in range(B)]
    )
    sigma = np.stack(
        [res.results[b]["sgT"].reshape(E, N).T.astype(np.float32) for b in range(B)]
    )
    return (mu, sigma), res


def kernel(x, W_mu, b_mu, W_logvar, b_logvar):
    (mu, sigma), _ = run(x, W_mu, b_mu, W_logvar, b_logvar, trace=False)
    return mu, sigma
